# revision 3
# baseline (speedup 1.0000x reference)
"""AttentionPooling (segment softmax-pool) Trainium2 kernel.

Graphs are sharded across 8 cores (1024 graphs each, 8 windows of 128); a
window's nodes are host-padded to T and processed in 512-node groups.

out[g] = (sum_{n in g} e_n x_n) / (sum_n e_n + eps),
e_n = exp(tanh(x_n W1 + b1) W2 + b2).

Key layout/precision choices (vs an all-bf16 dual-layout baseline):
  * mm1 (h^T = W1^T x^T) in fp8e4m3 DoubleRow: x^T shipped fp8 (values x8),
    W1 const fp8 (x16, contiguous (kt, m) pair blocks for dual-fp8
    ldweights); the 1/128 dequant rides the tanh scale.  Halves the x^T DMA.
  * mm2 (logits) bf16: ht stationary per node-tile, W2 moving; logits land
    node-on-partition so exp/S-build stay cheap.
  * exp batched over group pairs ([128, 8] per 2 groups).
  * S[node, graph] = (iota == batch_rel) * e via one fused DVE tensor_scalar
    per 128-node tile; seg matmul (bf16) accumulates
    psum[graph, 0:257] += S^T @ [x | 1] over the window, then one divide +
    DMA per window.
  * Deep software pipeline over flattened (window, group) steps: at step i
    PE runs seg(i-5), mm1(i), mm2(i-3), so tanh/exp/S-build latency hides
    under PE streaming; windows prefetched one ahead (x^T before xn).
"""
import os
import sys

for _p in ("/opt/trn_rl_repo", "/root/.axon_site/_ro/trn_rl_repo"):
    if os.path.isdir(_p) and _p not in sys.path:
        sys.path.insert(0, _p)

import numpy as np
import ml_dtypes

import concourse.bacc as bacc
import concourse.tile as tile
from concourse import mybir
from concourse.bass_utils import run_bass_kernel_spmd

F32 = mybir.dt.float32
BF16 = mybir.dt.bfloat16
F8 = mybir.dt.float8e4
BF = ml_dtypes.bfloat16
F8NP = ml_dtypes.float8_e4m3fn

N_GRAPHS = 8192
HIDDEN = 256
CORES = 8
WPC = 8            # windows per core
WG = 128           # graphs per window
GRP = 512          # nodes per group
ROW = 258          # xn row: 256 x + 1.0 + pad
EPS = 1e-8
XSCALE = 8.0       # x quantization scale for the mm1 path
W1SCALE = 16.0
W2SCALE = 16.0

import os as _os
HT_FP8 = _os.environ.get("KV_HT_FP8", "0") == "1"
FIRST_CHUNKS = int(_os.environ.get("KV_FIRST_CHUNKS", "1"))
WARM_TABLE = _os.environ.get("KV_WARM", "0") == "1"
FIRST_XN_ACT = _os.environ.get("KV_XN_ACT", "0") == "1"
SEG_LAG = int(_os.environ.get("KV_SEG_LAG", "5"))
MM2_LAG = int(_os.environ.get("KV_MM2_LAG", "3"))
EXPQ = int(_os.environ.get("KV_EXPQ", "2"))
FIN_ACT = _os.environ.get("KV_FIN_ACT", "0") == "1"
BALANCE = _os.environ.get("KV_BALANCE", "1") == "1"
PREFETCH_G = int(_os.environ.get("KV_PREFETCH_G", "2"))
OUT_ACT = _os.environ.get("KV_OUT_ACT", "0") == "1"
INTERLEAVE = _os.environ.get("KV_ILV", "1") == "1"
ALL_XN_ACT = _os.environ.get("KV_ALL_XN_ACT", "0") == "1"


def _build_program(T_tiles: int, reps: int = 1, variant: str = "full",
                   zero_bias: bool = False, ht_fp8: bool = HT_FP8):
    n_full = T_tiles // 4
    tail = T_tiles % 4                      # node-tiles in the tail group
    gsz_list = [GRP] * n_full + ([128 * tail] if tail else [])
    ng = len(gsz_list)
    xtbase = [1024 * g for g in range(ng)]  # fp8 elems per partition
    cols = T_tiles
    XNW = T_tiles * ROW
    XTW = 2 * 128 * T_tiles

    nc = bacc.Bacc("TRN2", target_bir_lowering=False, debug=False,
                   num_devices=CORES)
    xn = nc.dram_tensor("xn", [WPC, 128, XNW], BF16, kind="ExternalInput").ap()
    xt = nc.dram_tensor("xt", [WPC, 128, XTW], F8, kind="ExternalInput").ap()
    br = nc.dram_tensor("br", [WPC, 128, cols], F32, kind="ExternalInput").ap()
    # W1 as [mb, kt, m] per partition (contiguous (kt, m) pair blocks for
    # dual-fp8 ldweights); W2 as adjacent (kt) pairs.
    cw1 = nc.dram_tensor("cw1", [128, 2, 2, 128], F8, kind="ExternalInput").ap()
    cw2 = nc.dram_tensor("cw2", [128, 2, 1], F8, kind="ExternalInput").ap()
    cbb = nc.dram_tensor("cbb", [128, 130], BF16, kind="ExternalInput").ap()
    cf = nc.dram_tensor("cf", [128, 3], F32, kind="ExternalInput").ap()
    out = nc.dram_tensor("out", [WPC * WG, HIDDEN], F32, kind="ExternalOutput").ap()

    HT_DT = F8 if ht_fp8 else BF16

    from contextlib import ExitStack
    with tile.TileContext(nc) as tc:
        with ExitStack() as ctx:
            cpool = ctx.enter_context(tc.tile_pool(name="const", bufs=1))
            brpool = ctx.enter_context(tc.tile_pool(name="brp", bufs=3))
            xnpool = ctx.enter_context(tc.tile_pool(name="xnp", bufs=3))
            xtpool = ctx.enter_context(tc.tile_pool(name="xtp", bufs=3))
            htpool = ctx.enter_context(tc.tile_pool(name="htp", bufs=6))
            etpool = ctx.enter_context(tc.tile_pool(name="etp", bufs=8))
            spool = ctx.enter_context(tc.tile_pool(name="sp", bufs=24 + 4 * max(0, EXPQ - 2)))
            owpool = ctx.enter_context(tc.tile_pool(name="ow", bufs=2))
            phpool = ctx.enter_context(tc.tile_pool(name="ph", bufs=2, space="PSUM"))
            plpool = ctx.enter_context(tc.tile_pool(name="pl", bufs=2, space="PSUM"))
            pgpool = ctx.enter_context(tc.tile_pool(name="pg", bufs=2, space="PSUM"))
            if reps > 1:
                ctx.enter_context(tc.For_i(0, reps, 1))

            c81 = cpool.tile([128, 2, 2, 128], F8)
            c82 = cpool.tile([128, 2, 1], F8)
            cb = cpool.tile([128, 130], BF16)
            cft = cpool.tile([128, 3], F32)

            def load_consts():
                nc.sync.dma_start(out=c81[:], in_=cw1[:])
                nc.sync.dma_start(out=c82[:], in_=cw2[:])
                nc.sync.dma_start(out=cb[:], in_=cbb[:])
                nc.sync.dma_start(out=cft[:], in_=cf[:])
            iota = cb[:, 0:128]
            if WARM_TABLE:
                warm = cpool.tile([128, 1], F32)
                nc.scalar.activation(warm[:], cft[:, 0:1],
                                     mybir.ActivationFunctionType.Tanh,
                                     bias=0.0, scale=1.0)

            wstate = {}

            def load_window(w, xn_chunks=1, xn_eng=None):
                # xt first: mm1 needs it immediately; xn only at seg lag.
                # xn_eng lets window 0's xn ride the (idle) ACT hwdge queue
                # so it streams concurrently with xt on the SP queue.
                brw = brpool.tile([128, cols], F32)
                nc.sync.dma_start(out=brw[:], in_=br[w])
                xtwt = xtpool.tile([128, XTW], F8)
                nc.sync.dma_start(out=xtwt[:], in_=xt[w])
                eng = xn_eng or (nc.scalar if ALL_XN_ACT else nc.sync)
                xnc = None
                xnwt = xnpool.tile([128, XNW], BF16)
                eng.dma_start(out=xnwt[:], in_=xn[w])
                pseg = pgpool.tile([128, 257], F32)
                wstate[w] = dict(brw=brw, xnwt=xnwt, xnc=xnc, xtwt=xtwt,
                                 pseg=pseg)

            def xn_slice(w, g, t):
                ws = wstate[w]
                base = (g * 4 + t) * ROW
                if ws["xnc"] is not None:
                    step = XNW // len(ws["xnc"])
                    c, off = base // step, base % step
                    return ws["xnc"][c][:, off:off + 257]
                return ws["xnwt"][:, base:base + 257]

            if variant == "dma":
                for w in range(WPC):
                    load_window(w)
                    ws = wstate[w]
                    for nm in ("xnwt", "brw"):
                        dum = etpool.tile([128, 1], F32)
                        nc.vector.tensor_scalar(dum[:], ws[nm][:, 0:1], 1.0,
                                                None, op0=mybir.AluOpType.mult)
                    dum2 = etpool.tile([128, 1], F32)
                    nc.vector.tensor_scalar(dum2[:], ws["xtwt"][:, 0:1],
                                            1.0, None, op0=mybir.AluOpType.mult)

            steps = [] if variant in ("dma", "nop") else \
                [(w, g) for w in range(WPC) for g in range(ng)]
            gstate = {}
            pairstate = {}

            def emit_mm1_mm(i, m):
                w, g = steps[i]
                gsz = gsz_list[g]
                ws = wstate[w]
                if m == 0:
                    ph = phpool.tile([128, 2, GRP], F32)
                    gstate[i] = dict(ph=ph)
                ph = gstate[i]["ph"]
                xtg = ws["xtwt"][:, xtbase[g]:xtbase[g] + 2 * gsz].rearrange(
                    "p (k n) -> p k n", k=2)
                nc.tensor.matmul(ph[:, m, 0:gsz],
                                 c81[:, m],
                                 xtg,
                                 start=True, stop=True,
                                 perf_mode=mybir.MatmulPerfMode.DoubleRow)

            def emit_mm1_tanh(i, mm=True):
                w, g = steps[i]
                gsz = gsz_list[g]
                ws = wstate[w]
                if mm:
                    emit_mm1_mm(i, 0)
                    emit_mm1_mm(i, 1)
                ph = gstate[i]["ph"]
                hsc = 1.0 / (XSCALE * W1SCALE)
                if ht_fp8:
                    # t-major storage so mm2's dual-fp8 lhsT is contiguous
                    ht = htpool.tile([128, 4, 2, 128], HT_DT)
                    ht_w = ht[:].rearrange("p t k n -> p k t n")
                    ph_r = ph[:].rearrange("p k (t n) -> p k t n", t=4)
                    if zero_bias:
                        nc.scalar.activation(ht_w, ph_r,
                                             mybir.ActivationFunctionType.Tanh,
                                             bias=0.0, scale=hsc)
                    else:
                        for m in range(2):
                            nc.scalar.activation(ht_w[:, m], ph_r[:, m],
                                                 mybir.ActivationFunctionType.Tanh,
                                                 bias=cft[:, m:m + 1], scale=hsc)
                else:
                    ht = htpool.tile([128, 2, GRP], HT_DT)
                    if zero_bias:
                        nc.scalar.activation(ht[:, :, 0:gsz], ph[:, :, 0:gsz],
                                             mybir.ActivationFunctionType.Tanh,
                                             bias=0.0, scale=hsc)
                    else:
                        for m in range(2):
                            nc.scalar.activation(ht[:, m, 0:gsz],
                                                 ph[:, m, 0:gsz],
                                                 mybir.ActivationFunctionType.Tanh,
                                                 bias=cft[:, m:m + 1], scale=hsc)
                gstate[i]["ht"] = ht

            def build_s(j, et_ap, base):
                wj, gj = steps[j]
                wsj = wstate[wj]
                sts = []
                for t in range(gsz_list[gj] // 128):
                    st = spool.tile([128, 128], BF16)
                    eng = nc.vector
                    eng.tensor_scalar(st[:], iota,
                                      wsj["brw"][:, gj * 4 + t:gj * 4 + t + 1],
                                      et_ap[:, base + t:base + t + 1],
                                      op0=mybir.AluOpType.is_equal,
                                      op1=mybir.AluOpType.mult)
                    sts.append(st)
                gstate[j]["sts"] = sts

            def emit_mm2_exp_s(i):
                w, g = steps[i]
                gs = gstate[i]
                ht = gs["ht"]
                q = i % EXPQ
                if q == 0:
                    pl = plpool.tile([128, 4 * EXPQ], F32)
                    pairstate[i] = pl
                else:
                    pl = pairstate[i - q]
                    if q == EXPQ - 1:
                        pairstate.pop(i - q)
                lo = 4 * q
                tcnt = gsz_list[g] // 128
                if ht_fp8:
                    for t in range(tcnt):
                        nc.tensor.matmul(pl[:, lo + t:lo + t + 1],
                                         ht[:, t],
                                         c82[:],
                                         start=True, stop=True,
                                         perf_mode=mybir.MatmulPerfMode.DoubleRow)
                else:
                    for t in range(tcnt):
                        for k in range(2):
                            nc.tensor.matmul(pl[:, lo + t:lo + t + 1],
                                             ht[:, k, 128 * t:128 * (t + 1)],
                                             cb[:, 128 + k:129 + k],
                                             start=(k == 0), stop=(k == 1))
                if q == EXPQ - 1 or i == len(steps) - 1:
                    esc = 1.0 / W2SCALE
                    ebias = 0.0 if zero_bias else cft[:, 2:3]
                    et = etpool.tile([128, 4 * EXPQ], F32)
                    span = 4 * q + tcnt
                    nc.scalar.activation(et[:, 0:span], pl[:, 0:span],
                                         mybir.ActivationFunctionType.Exp,
                                         bias=ebias, scale=esc)
                    for j in range(i - q, i + 1):
                        build_s(j, et, 4 * (j - (i - q)))

            def emit_seg(i):
                w, g = steps[i]
                tcnt = gsz_list[g] // 128
                ws = wstate[w]
                gs = gstate.pop(i)
                for t in range(tcnt):
                    nc.tensor.matmul(ws["pseg"][:],
                                     gs["sts"][t][:],
                                     xn_slice(w, g, t),
                                     start=(g == 0 and t == 0),
                                     stop=(g == ng - 1 and t == tcnt - 1))
                if g == ng - 1:
                    finalize_window(w)

            def finalize_window(w):
                ws = wstate.pop(w)
                pseg = ws["pseg"]
                dtmp = owpool.tile([128, 1], F32)
                nc.vector.tensor_scalar_add(dtmp[:], pseg[:, 256:257], EPS)
                rec = owpool.tile([128, 1], F32)
                nc.vector.reciprocal(rec[:], dtmp[:])
                ow = owpool.tile([128, HIDDEN], F32)
                if FIN_ACT:
                    nc.scalar.activation(ow[:], pseg[:, 0:256],
                                         mybir.ActivationFunctionType.Copy,
                                         bias=0.0, scale=rec[:])
                else:
                    nc.vector.tensor_scalar(ow[:], pseg[:, 0:256], rec[:],
                                            None, op0=mybir.AluOpType.mult)
                oeng = nc.scalar if OUT_ACT else nc.sync
                oeng.dma_start(out=out[w * WG:(w + 1) * WG, :], in_=ow[:])

            if variant == "nop":
                dnp = owpool.tile([128, 1], F32)
                nc.vector.tensor_scalar(dnp[:], cft[:, 0:1], 1.0, None,
                                        op0=mybir.AluOpType.mult)

            load_consts()
            if steps:
                load_window(0, xn_chunks=FIRST_CHUNKS,
                            xn_eng=nc.scalar if FIRST_XN_ACT else None)
            n = len(steps)
            lag = SEG_LAG
            m2lag = MM2_LAG
            for i in range(n + lag if n else 0):
                if i < n:
                    w, g = steps[i]
                    if g == PREFETCH_G and w + 1 < WPC:
                        load_window(w + 1)
                if INTERLEAVE:
                    if i >= lag:
                        emit_seg(i - lag)
                    if i < n:
                        emit_mm1_mm(i, 0)
                    if m2lag <= i < n + m2lag:
                        emit_mm2_exp_s(i - m2lag)
                    if i < n:
                        emit_mm1_mm(i, 1)
                        emit_mm1_tanh(i, mm=False)
                else:
                    if i >= lag:
                        emit_seg(i - lag)
                    if i < n:
                        emit_mm1_tanh(i)
                    if m2lag <= i < n + m2lag:
                        emit_mm2_exp_s(i - m2lag)
    nc.compile()
    return nc


def _prep_inputs(x, batch, W1, b1, W2, b2):
    batch = np.asarray(batch).astype(np.int64)
    x = np.asarray(x, dtype=np.float32)

    gstarts = np.searchsorted(batch, np.arange(0, N_GRAPHS + 1))
    gsizes = np.diff(gstarts)

    # Assign graphs to windows within each core, balancing node counts
    # (LPT greedy) so the global max window -- and with it T -- shrinks.
    perm = np.zeros((CORES, WPC, WG), np.int64)
    wmax = 0
    for c in range(CORES):
        ids = np.arange(c * WPC * WG, (c + 1) * WPC * WG)
        if BALANCE:
            order = ids[np.argsort(-gsizes[ids], kind="stable")]
            loads = [0] * WPC
            counts = [0] * WPC
            buckets = [[] for _ in range(WPC)]
            for gid in order:
                cand = [i for i in range(WPC) if counts[i] < WG]
                wsel = min(cand, key=lambda i: loads[i])
                buckets[wsel].append(gid)
                loads[wsel] += int(gsizes[gid])
                counts[wsel] += 1
            for w in range(WPC):
                perm[c, w] = np.sort(np.array(buckets[w], np.int64))
            wmax = max(wmax, max(loads))
        else:
            perm[c] = ids.reshape(WPC, WG)
            for w in range(WPC):
                wmax = max(wmax, int(gsizes[perm[c, w]].sum()))

    T_tiles = max(4, (int(wmax) + 127) // 128)
    n_full = T_tiles // 4
    tail = T_tiles % 4
    gsz_list = [GRP] * n_full + ([128 * tail] if tail else [])
    T = T_tiles * 128
    cols = T_tiles
    XNW = T_tiles * ROW
    XTW = 2 * 128 * T_tiles

    xbf = x.astype(BF)
    x8 = (x * XSCALE).astype(F8NP)

    W1 = np.asarray(W1, np.float32)
    W2 = np.asarray(W2, np.float32).reshape(-1)
    cw1 = np.zeros((128, 2, 2, 128), dtype=F8NP)
    cw2 = np.zeros((128, 2, 1), dtype=F8NP)
    for kt in range(2):
        for mb in range(2):
            cw1[:, mb, kt, :] = (W1[kt * 128:(kt + 1) * 128,
                                    mb * 128:(mb + 1) * 128] * W1SCALE).astype(F8NP)
        cw2[:, kt, 0] = (W2[kt * 128:(kt + 1) * 128] * W2SCALE).astype(F8NP)
    cbb = np.zeros((128, 130), dtype=BF)
    cbb[:, 0:128] = np.tile(np.arange(128, dtype=np.float32), (128, 1)).astype(BF)
    for k in range(2):
        cbb[:, 128 + k] = (W2[128 * k:128 * (k + 1)] * W2SCALE).astype(BF)

    cf = np.zeros((128, 3), dtype=np.float32)
    cf[:, 0] = np.asarray(b1, np.float32)[0:128]
    cf[:, 1] = np.asarray(b1, np.float32)[128:256]
    cf[:, 2] = float(np.asarray(b2, np.float32).reshape(-1)[0])
    zero_bias = bool((np.asarray(b1) == 0).all() and (np.asarray(b2) == 0).all())

    in_maps = []
    for c in range(CORES):
        xn_c = np.zeros((WPC, T, ROW), dtype=BF)
        xt_lin = np.zeros((WPC, 2, 128, T), dtype=F8NP)
        br_c = np.full((WPC, 128, cols), -1.0, dtype=np.float32)
        for w in range(WPC):
            gl = perm[c, w]
            sz = int(gsizes[gl].sum())
            if sz:
                idx = np.concatenate(
                    [np.arange(gstarts[g], gstarts[g + 1]) for g in gl])
                xn_c[w, :sz, 0:256] = xbf[idx]
                xn_c[w, :sz, 256] = BF(1.0)
                xt_lin[w, 0, :, :sz] = x8[idx, 0:128].T
                xt_lin[w, 1, :, :sz] = x8[idx, 128:256].T
                tmp = np.full(T, -1.0, dtype=np.float32)
                tmp[:sz] = np.repeat(
                    np.arange(WG, dtype=np.float32), gsizes[gl])
                br_c[w] = tmp.reshape(cols, 128).T
        # xn swizzle: [w, tile*128+p, d] -> [w, p, tile*ROW + d]
        xn_sw = np.ascontiguousarray(
            xn_c.reshape(WPC, T_tiles, 128, ROW).transpose(0, 2, 1, 3)
        ).reshape(WPC, 128, XNW)
        # xt flat: per group [kt, 128, gsz] -> [128, 2*gsz] at base 1024*g
        xt_sw = np.zeros((WPC, 128, XTW), dtype=F8NP)
        for g, gsz in enumerate(gsz_list):
            s0 = g * GRP
            blk = xt_lin[:, :, :, s0:s0 + gsz]          # [WPC, 2, 128, gsz]
            xt_sw[:, :, 1024 * g:1024 * g + 2 * gsz] = np.ascontiguousarray(
                blk.transpose(0, 2, 1, 3)).reshape(WPC, 128, 2 * gsz)
        in_maps.append(dict(xn=xn_sw, xt=xt_sw, br=br_c, cw1=cw1, cw2=cw2,
                            cbb=cbb, cf=cf))
    return T_tiles, in_maps, zero_bias, perm


_PROGRAM_CACHE = {}


def kernel(x, batch, W1, b1, W2, b2):
    T_tiles, in_maps, zb, perm = _prep_inputs(x, batch, W1, b1, W2, b2)
    key = (T_tiles, zb)
    if key not in _PROGRAM_CACHE:
        _PROGRAM_CACHE[key] = _build_program(T_tiles, zero_bias=zb)
    nc = _PROGRAM_CACHE[key]
    res = run_bass_kernel_spmd(nc, in_maps, list(range(CORES))).results
    raw = np.concatenate([res[c]["out"] for c in range(CORES)], axis=0)
    final = np.empty_like(raw)
    final[perm.reshape(-1)] = raw
    return final



# revision 7
# speedup vs baseline: 1.0459x; 1.0459x over previous
"""AttentionPooling (segment softmax-pool) Trainium2 kernel.

Graphs are sharded across 8 cores (1024 graphs each, 8 windows of 128); a
window's nodes are host-padded to T and processed in 512-node groups.

out[g] = (sum_{n in g} e_n x_n) / (sum_n e_n + eps),
e_n = exp(tanh(x_n W1 + b1) W2 + b2).

Key layout/precision choices (vs an all-bf16 dual-layout baseline):
  * mm1 (h^T = W1^T x^T) in fp8e4m3 DoubleRow: x^T shipped fp8 (values x8),
    W1 const fp8 (x16, contiguous (kt, m) pair blocks for dual-fp8
    ldweights); the 1/128 dequant rides the tanh scale.  Halves the x^T DMA.
  * mm2 (logits) bf16: ht stationary per node-tile, W2 moving; logits land
    node-on-partition so exp/S-build stay cheap.
  * exp batched over group pairs ([128, 8] per 2 groups).
  * S[node, graph] = (iota == batch_rel) * e via one fused DVE tensor_scalar
    per 128-node tile; seg matmul (bf16) accumulates
    psum[graph, 0:257] += S^T @ [x | 1] over the window, then one divide +
    DMA per window.
  * Deep software pipeline over flattened (window, group) steps: at step i
    PE runs seg(i-5), mm1(i), mm2(i-3), so tanh/exp/S-build latency hides
    under PE streaming; windows prefetched one ahead (x^T before xn).
"""
import os
import sys

for _p in ("/opt/trn_rl_repo", "/root/.axon_site/_ro/trn_rl_repo"):
    if os.path.isdir(_p) and _p not in sys.path:
        sys.path.insert(0, _p)

import numpy as np
import ml_dtypes

import concourse.bacc as bacc
import concourse.tile as tile
from concourse import mybir
from concourse.bass_utils import run_bass_kernel_spmd

F32 = mybir.dt.float32
BF16 = mybir.dt.bfloat16
F8 = mybir.dt.float8e4
BF = ml_dtypes.bfloat16
F8NP = ml_dtypes.float8_e4m3fn

N_GRAPHS = 8192
HIDDEN = 256
CORES = 8
WPC = 8            # windows per core
WG = 128           # graphs per window
GRP = 512          # nodes per group
ROW = 258          # xn row: 256 x + 1.0 + pad
EPS = 1e-8
XSCALE = 8.0       # x quantization scale for the mm1 path
W1SCALE = 16.0
W2SCALE = 16.0

import os as _os
HT_FP8 = _os.environ.get("KV_HT_FP8", "0") == "1"
FIRST_CHUNKS = int(_os.environ.get("KV_FIRST_CHUNKS", "1"))
WARM_TABLE = _os.environ.get("KV_WARM", "0") == "1"
FIRST_XN_ACT = _os.environ.get("KV_XN_ACT", "0") == "1"
SEG_LAG = int(_os.environ.get("KV_SEG_LAG", "5"))
MM2_LAG = int(_os.environ.get("KV_MM2_LAG", "3"))
EXPQ = int(_os.environ.get("KV_EXPQ", "2"))
FIN_ACT = _os.environ.get("KV_FIN_ACT", "0") == "1"
BALANCE = _os.environ.get("KV_BALANCE", "1") == "1"
PREFETCH_G = int(_os.environ.get("KV_PREFETCH_G", "2"))
OUT_ACT = _os.environ.get("KV_OUT_ACT", "0") == "1"
INTERLEAVE = _os.environ.get("KV_ILV", "1") == "1"
ALL_XN_ACT = _os.environ.get("KV_ALL_XN_ACT", "0") == "1"
XT2 = _os.environ.get("KV_XT2", "0") == "1"   # prefetch xt 2 windows ahead


def _build_program(T_tiles: int, reps: int = 1, variant: str = "full",
                   zero_bias: bool = False, ht_fp8: bool = HT_FP8):
    n_full = T_tiles // 4
    tail = T_tiles % 4                      # node-tiles in the tail group
    gsz_list = [GRP] * n_full + ([128 * tail] if tail else [])
    ng = len(gsz_list)
    xtbase = [1024 * g for g in range(ng)]  # fp8 elems per partition
    cols = T_tiles
    XNW = T_tiles * ROW
    XTW = 2 * 128 * T_tiles

    nc = bacc.Bacc("TRN2", target_bir_lowering=False, debug=False,
                   num_devices=CORES)
    xn = nc.dram_tensor("xn", [WPC, 128, XNW], BF16, kind="ExternalInput").ap()
    xt = nc.dram_tensor("xt", [WPC, 128, XTW], F8, kind="ExternalInput").ap()
    br = nc.dram_tensor("br", [WPC, 128, cols], F32, kind="ExternalInput").ap()
    # W1 as [mb, kt, m] per partition (contiguous (kt, m) pair blocks for
    # dual-fp8 ldweights); W2 as adjacent (kt) pairs.
    cw1 = nc.dram_tensor("cw1", [128, 2, 2, 128], F8, kind="ExternalInput").ap()
    cw2 = nc.dram_tensor("cw2", [128, 2, 1], F8, kind="ExternalInput").ap()
    cbb = nc.dram_tensor("cbb", [128, 130], BF16, kind="ExternalInput").ap()
    cf = nc.dram_tensor("cf", [128, 3], F32, kind="ExternalInput").ap()
    out = nc.dram_tensor("out", [WPC * WG, HIDDEN], F32, kind="ExternalOutput").ap()

    HT_DT = F8 if ht_fp8 else BF16

    from contextlib import ExitStack
    with tile.TileContext(nc) as tc:
        with ExitStack() as ctx:
            cpool = ctx.enter_context(tc.tile_pool(name="const", bufs=1))
            brpool = ctx.enter_context(tc.tile_pool(name="brp", bufs=4 if XT2 else 3))
            xnpool = ctx.enter_context(tc.tile_pool(name="xnp", bufs=3))
            xtpool = ctx.enter_context(tc.tile_pool(name="xtp", bufs=4 if XT2 else 3))
            htpool = ctx.enter_context(tc.tile_pool(name="htp", bufs=6))
            etpool = ctx.enter_context(tc.tile_pool(name="etp", bufs=8))
            spool = ctx.enter_context(tc.tile_pool(name="sp", bufs=24 + 4 * max(0, EXPQ - 2)))
            owpool = ctx.enter_context(tc.tile_pool(name="ow", bufs=2))
            phpool = ctx.enter_context(tc.tile_pool(name="ph", bufs=2, space="PSUM"))
            plpool = ctx.enter_context(tc.tile_pool(name="pl", bufs=2, space="PSUM"))
            pgpool = ctx.enter_context(tc.tile_pool(name="pg", bufs=2, space="PSUM"))
            if reps > 1:
                ctx.enter_context(tc.For_i(0, reps, 1))

            c81 = cpool.tile([128, 2, 2, 128], F8)
            c82 = cpool.tile([128, 2, 1], F8)
            cb = cpool.tile([128, 130], BF16)
            cft = cpool.tile([128, 3], F32)

            def load_consts():
                nc.sync.dma_start(out=c81[:], in_=cw1[:])
                nc.sync.dma_start(out=c82[:], in_=cw2[:])
                nc.sync.dma_start(out=cb[:], in_=cbb[:])
                nc.sync.dma_start(out=cft[:], in_=cf[:])
            iota = cb[:, 0:128]
            if WARM_TABLE:
                warm = cpool.tile([128, 1], F32)
                nc.scalar.activation(warm[:], cft[:, 0:1],
                                     mybir.ActivationFunctionType.Tanh,
                                     bias=0.0, scale=1.0)

            wstate = {}

            def load_xt_br(w):
                brw = brpool.tile([128, cols], F32)
                nc.sync.dma_start(out=brw[:], in_=br[w])
                xtwt = xtpool.tile([128, XTW], F8)
                nc.sync.dma_start(out=xtwt[:], in_=xt[w])
                wstate[w] = dict(brw=brw, xtwt=xtwt, xnc=None)

            def load_xn_w(w, xn_eng=None):
                eng = xn_eng or (nc.scalar if ALL_XN_ACT else nc.sync)
                xnwt = xnpool.tile([128, XNW], BF16)
                eng.dma_start(out=xnwt[:], in_=xn[w])
                pseg = pgpool.tile([128, 257], F32)
                wstate[w].update(xnwt=xnwt, pseg=pseg)

            def load_window(w, xn_chunks=1, xn_eng=None):
                # xt first: mm1 needs it immediately; xn only at seg lag.
                # xn_eng lets window 0's xn ride the (idle) ACT hwdge queue
                # so it streams concurrently with xt on the SP queue.
                load_xt_br(w)
                load_xn_w(w, xn_eng=xn_eng)

            def xn_slice(w, g, t):
                ws = wstate[w]
                base = (g * 4 + t) * ROW
                if ws["xnc"] is not None:
                    step = XNW // len(ws["xnc"])
                    c, off = base // step, base % step
                    return ws["xnc"][c][:, off:off + 257]
                return ws["xnwt"][:, base:base + 257]

            if variant == "dma":
                for w in range(WPC):
                    load_window(w)
                    ws = wstate[w]
                    for nm in ("xnwt", "brw"):
                        dum = etpool.tile([128, 1], F32)
                        nc.vector.tensor_scalar(dum[:], ws[nm][:, 0:1], 1.0,
                                                None, op0=mybir.AluOpType.mult)
                    dum2 = etpool.tile([128, 1], F32)
                    nc.vector.tensor_scalar(dum2[:], ws["xtwt"][:, 0:1],
                                            1.0, None, op0=mybir.AluOpType.mult)

            steps = [] if variant in ("dma", "nop") else \
                [(w, g) for w in range(WPC) for g in range(ng)]
            gstate = {}
            pairstate = {}

            def emit_mm1_mm(i, m):
                w, g = steps[i]
                gsz = gsz_list[g]
                ws = wstate[w]
                if m == 0:
                    ph = phpool.tile([128, 2, GRP], F32)
                    gstate[i] = dict(ph=ph)
                ph = gstate[i]["ph"]
                xtg = ws["xtwt"][:, xtbase[g]:xtbase[g] + 2 * gsz].rearrange(
                    "p (k n) -> p k n", k=2)
                nc.tensor.matmul(ph[:, m, 0:gsz],
                                 c81[:, m],
                                 xtg,
                                 start=True, stop=True,
                                 perf_mode=mybir.MatmulPerfMode.DoubleRow)

            def emit_mm1_tanh(i, mm=True):
                w, g = steps[i]
                gsz = gsz_list[g]
                ws = wstate[w]
                if mm:
                    emit_mm1_mm(i, 0)
                    emit_mm1_mm(i, 1)
                ph = gstate[i]["ph"]
                hsc = 1.0 / (XSCALE * W1SCALE)
                if ht_fp8:
                    # t-major storage so mm2's dual-fp8 lhsT is contiguous
                    ht = htpool.tile([128, 4, 2, 128], HT_DT)
                    ht_w = ht[:].rearrange("p t k n -> p k t n")
                    ph_r = ph[:].rearrange("p k (t n) -> p k t n", t=4)
                    if zero_bias:
                        nc.scalar.activation(ht_w, ph_r,
                                             mybir.ActivationFunctionType.Tanh,
                                             bias=0.0, scale=hsc)
                    else:
                        for m in range(2):
                            nc.scalar.activation(ht_w[:, m], ph_r[:, m],
                                                 mybir.ActivationFunctionType.Tanh,
                                                 bias=cft[:, m:m + 1], scale=hsc)
                else:
                    ht = htpool.tile([128, 2, GRP], HT_DT)
                    if zero_bias:
                        nc.scalar.activation(ht[:, :, 0:gsz], ph[:, :, 0:gsz],
                                             mybir.ActivationFunctionType.Tanh,
                                             bias=0.0, scale=hsc)
                    else:
                        for m in range(2):
                            nc.scalar.activation(ht[:, m, 0:gsz],
                                                 ph[:, m, 0:gsz],
                                                 mybir.ActivationFunctionType.Tanh,
                                                 bias=cft[:, m:m + 1], scale=hsc)
                gstate[i]["ht"] = ht

            def build_s(j, et_ap, base):
                wj, gj = steps[j]
                wsj = wstate[wj]
                sts = []
                for t in range(gsz_list[gj] // 128):
                    st = spool.tile([128, 128], BF16)
                    eng = nc.vector
                    eng.tensor_scalar(st[:], iota,
                                      wsj["brw"][:, gj * 4 + t:gj * 4 + t + 1],
                                      et_ap[:, base + t:base + t + 1],
                                      op0=mybir.AluOpType.is_equal,
                                      op1=mybir.AluOpType.mult)
                    sts.append(st)
                gstate[j]["sts"] = sts

            def emit_mm2_exp_s(i):
                w, g = steps[i]
                gs = gstate[i]
                ht = gs["ht"]
                q = i % EXPQ
                if q == 0:
                    pl = plpool.tile([128, 4 * EXPQ], F32)
                    pairstate[i] = pl
                else:
                    pl = pairstate[i - q]
                    if q == EXPQ - 1:
                        pairstate.pop(i - q)
                lo = 4 * q
                tcnt = gsz_list[g] // 128
                if ht_fp8:
                    for t in range(tcnt):
                        nc.tensor.matmul(pl[:, lo + t:lo + t + 1],
                                         ht[:, t],
                                         c82[:],
                                         start=True, stop=True,
                                         perf_mode=mybir.MatmulPerfMode.DoubleRow)
                else:
                    for t in range(tcnt):
                        for k in range(2):
                            nc.tensor.matmul(pl[:, lo + t:lo + t + 1],
                                             ht[:, k, 128 * t:128 * (t + 1)],
                                             cb[:, 128 + k:129 + k],
                                             start=(k == 0), stop=(k == 1))
                if q == EXPQ - 1 or i == len(steps) - 1:
                    esc = 1.0 / W2SCALE
                    ebias = 0.0 if zero_bias else cft[:, 2:3]
                    et = etpool.tile([128, 4 * EXPQ], F32)
                    span = 4 * q + tcnt
                    nc.scalar.activation(et[:, 0:span], pl[:, 0:span],
                                         mybir.ActivationFunctionType.Exp,
                                         bias=ebias, scale=esc)
                    for j in range(i - q, i + 1):
                        build_s(j, et, 4 * (j - (i - q)))

            def emit_seg(i):
                w, g = steps[i]
                tcnt = gsz_list[g] // 128
                ws = wstate[w]
                gs = gstate.pop(i)
                for t in range(tcnt):
                    nc.tensor.matmul(ws["pseg"][:],
                                     gs["sts"][t][:],
                                     xn_slice(w, g, t),
                                     start=(g == 0 and t == 0),
                                     stop=(g == ng - 1 and t == tcnt - 1))
                if g == ng - 1:
                    finalize_window(w)

            def finalize_window(w):
                ws = wstate.pop(w)
                pseg = ws["pseg"]
                dtmp = owpool.tile([128, 1], F32)
                nc.vector.tensor_scalar_add(dtmp[:], pseg[:, 256:257], EPS)
                rec = owpool.tile([128, 1], F32)
                nc.vector.reciprocal(rec[:], dtmp[:])
                ow = owpool.tile([128, HIDDEN], F32)
                if FIN_ACT:
                    nc.scalar.activation(ow[:], pseg[:, 0:256],
                                         mybir.ActivationFunctionType.Copy,
                                         bias=0.0, scale=rec[:])
                else:
                    nc.vector.tensor_scalar(ow[:], pseg[:, 0:256], rec[:],
                                            None, op0=mybir.AluOpType.mult)
                oeng = nc.scalar if OUT_ACT else nc.sync
                oeng.dma_start(out=out[w * WG:(w + 1) * WG, :], in_=ow[:])

            if variant == "nop":
                dnp = owpool.tile([128, 1], F32)
                nc.vector.tensor_scalar(dnp[:], cft[:, 0:1], 1.0, None,
                                        op0=mybir.AluOpType.mult)

            load_consts()
            if steps:
                load_window(0, xn_chunks=FIRST_CHUNKS,
                            xn_eng=nc.scalar if FIRST_XN_ACT else None)
                if XT2 and WPC > 1:
                    load_xt_br(1)
            n = len(steps)
            lag = SEG_LAG
            m2lag = MM2_LAG
            for i in range(n + lag if n else 0):
                if i < n:
                    w, g = steps[i]
                    if g == PREFETCH_G and w + 1 < WPC:
                        if XT2:
                            load_xn_w(w + 1)
                            if w + 2 < WPC:
                                load_xt_br(w + 2)
                        else:
                            load_window(w + 1)
                if INTERLEAVE:
                    if i >= lag:
                        emit_seg(i - lag)
                    if i < n:
                        emit_mm1_mm(i, 0)
                    if m2lag <= i < n + m2lag:
                        emit_mm2_exp_s(i - m2lag)
                    if i < n:
                        emit_mm1_mm(i, 1)
                        emit_mm1_tanh(i, mm=False)
                else:
                    if i >= lag:
                        emit_seg(i - lag)
                    if i < n:
                        emit_mm1_tanh(i)
                    if m2lag <= i < n + m2lag:
                        emit_mm2_exp_s(i - m2lag)
    nc.compile()
    return nc


def _prep_inputs(x, batch, W1, b1, W2, b2):
    batch = np.asarray(batch).astype(np.int64)
    x = np.asarray(x, dtype=np.float32)

    gstarts = np.searchsorted(batch, np.arange(0, N_GRAPHS + 1))
    gsizes = np.diff(gstarts)

    # Assign graphs to windows within each core, balancing node counts
    # (LPT greedy) so the global max window -- and with it T -- shrinks.
    perm = np.zeros((CORES, WPC, WG), np.int64)
    wmax = 0
    for c in range(CORES):
        ids = np.arange(c * WPC * WG, (c + 1) * WPC * WG)
        if BALANCE:
            order = ids[np.argsort(-gsizes[ids], kind="stable")]
            loads = [0] * WPC
            counts = [0] * WPC
            buckets = [[] for _ in range(WPC)]
            for gid in order:
                cand = [i for i in range(WPC) if counts[i] < WG]
                wsel = min(cand, key=lambda i: loads[i])
                buckets[wsel].append(gid)
                loads[wsel] += int(gsizes[gid])
                counts[wsel] += 1
            for w in range(WPC):
                perm[c, w] = np.sort(np.array(buckets[w], np.int64))
            wmax = max(wmax, max(loads))
        else:
            perm[c] = ids.reshape(WPC, WG)
            for w in range(WPC):
                wmax = max(wmax, int(gsizes[perm[c, w]].sum()))

    T_tiles = max(4, (int(wmax) + 127) // 128)
    n_full = T_tiles // 4
    tail = T_tiles % 4
    gsz_list = [GRP] * n_full + ([128 * tail] if tail else [])
    T = T_tiles * 128
    cols = T_tiles
    XNW = T_tiles * ROW
    XTW = 2 * 128 * T_tiles

    xbf = x.astype(BF)
    x8 = (x * XSCALE).astype(F8NP)

    W1 = np.asarray(W1, np.float32)
    W2 = np.asarray(W2, np.float32).reshape(-1)
    cw1 = np.zeros((128, 2, 2, 128), dtype=F8NP)
    cw2 = np.zeros((128, 2, 1), dtype=F8NP)
    for kt in range(2):
        for mb in range(2):
            cw1[:, mb, kt, :] = (W1[kt * 128:(kt + 1) * 128,
                                    mb * 128:(mb + 1) * 128] * W1SCALE).astype(F8NP)
        cw2[:, kt, 0] = (W2[kt * 128:(kt + 1) * 128] * W2SCALE).astype(F8NP)
    cbb = np.zeros((128, 130), dtype=BF)
    cbb[:, 0:128] = np.tile(np.arange(128, dtype=np.float32), (128, 1)).astype(BF)
    for k in range(2):
        cbb[:, 128 + k] = (W2[128 * k:128 * (k + 1)] * W2SCALE).astype(BF)

    cf = np.zeros((128, 3), dtype=np.float32)
    cf[:, 0] = np.asarray(b1, np.float32)[0:128]
    cf[:, 1] = np.asarray(b1, np.float32)[128:256]
    cf[:, 2] = float(np.asarray(b2, np.float32).reshape(-1)[0])
    zero_bias = bool((np.asarray(b1) == 0).all() and (np.asarray(b2) == 0).all())

    in_maps = []
    for c in range(CORES):
        xn_c = np.zeros((WPC, T, ROW), dtype=BF)
        xt_lin = np.zeros((WPC, 2, 128, T), dtype=F8NP)
        br_c = np.full((WPC, 128, cols), -1.0, dtype=np.float32)
        for w in range(WPC):
            gl = perm[c, w]
            sz = int(gsizes[gl].sum())
            if sz:
                idx = np.concatenate(
                    [np.arange(gstarts[g], gstarts[g + 1]) for g in gl])
                xn_c[w, :sz, 0:256] = xbf[idx]
                xn_c[w, :sz, 256] = BF(1.0)
                xt_lin[w, 0, :, :sz] = x8[idx, 0:128].T
                xt_lin[w, 1, :, :sz] = x8[idx, 128:256].T
                tmp = np.full(T, -1.0, dtype=np.float32)
                tmp[:sz] = np.repeat(
                    np.arange(WG, dtype=np.float32), gsizes[gl])
                br_c[w] = tmp.reshape(cols, 128).T
        # xn swizzle: [w, tile*128+p, d] -> [w, p, tile*ROW + d]
        xn_sw = np.ascontiguousarray(
            xn_c.reshape(WPC, T_tiles, 128, ROW).transpose(0, 2, 1, 3)
        ).reshape(WPC, 128, XNW)
        # xt flat: per group [kt, 128, gsz] -> [128, 2*gsz] at base 1024*g
        xt_sw = np.zeros((WPC, 128, XTW), dtype=F8NP)
        for g, gsz in enumerate(gsz_list):
            s0 = g * GRP
            blk = xt_lin[:, :, :, s0:s0 + gsz]          # [WPC, 2, 128, gsz]
            xt_sw[:, :, 1024 * g:1024 * g + 2 * gsz] = np.ascontiguousarray(
                blk.transpose(0, 2, 1, 3)).reshape(WPC, 128, 2 * gsz)
        in_maps.append(dict(xn=xn_sw, xt=xt_sw, br=br_c, cw1=cw1, cw2=cw2,
                            cbb=cbb, cf=cf))
    return T_tiles, in_maps, zero_bias, perm


_PROGRAM_CACHE = {}


def kernel(x, batch, W1, b1, W2, b2):
    T_tiles, in_maps, zb, perm = _prep_inputs(x, batch, W1, b1, W2, b2)
    key = (T_tiles, zb)
    if key not in _PROGRAM_CACHE:
        _PROGRAM_CACHE[key] = _build_program(T_tiles, zero_bias=zb)
    nc = _PROGRAM_CACHE[key]
    res = run_bass_kernel_spmd(nc, in_maps, list(range(CORES))).results
    raw = np.concatenate([res[c]["out"] for c in range(CORES)], axis=0)
    final = np.empty_like(raw)
    final[perm.reshape(-1)] = raw
    return final



# revision 11
# speedup vs baseline: 1.0593x; 1.0129x over previous
"""AttentionPooling (segment softmax-pool) Trainium2 kernel.

Graphs are sharded across 8 cores (1024 graphs each, 8 windows of 128); a
window's nodes are host-padded to T and processed in 512-node groups.

out[g] = (sum_{n in g} e_n x_n) / (sum_n e_n + eps),
e_n = exp(tanh(x_n W1 + b1) W2 + b2).

Key layout/precision choices (vs an all-bf16 dual-layout baseline):
  * mm1 (h^T = W1^T x^T) in fp8e4m3 DoubleRow: x^T shipped fp8 (values x8),
    W1 const fp8 (x16, contiguous (kt, m) pair blocks for dual-fp8
    ldweights); the 1/128 dequant rides the tanh scale.  Halves the x^T DMA.
  * mm2 (logits) bf16: ht stationary per node-tile, W2 moving; logits land
    node-on-partition so exp/S-build stay cheap.
  * exp batched over group pairs ([128, 8] per 2 groups).
  * S[node, graph] = (iota == batch_rel) * e via one fused DVE tensor_scalar
    per 128-node tile; seg matmul (bf16) accumulates
    psum[graph, 0:257] += S^T @ [x | 1] over the window, then one divide +
    DMA per window.
  * Deep software pipeline over flattened (window, group) steps: at step i
    PE runs seg(i-5), mm1(i), mm2(i-3), so tanh/exp/S-build latency hides
    under PE streaming; windows prefetched one ahead (x^T before xn).
"""
import os
import sys

for _p in ("/opt/trn_rl_repo", "/root/.axon_site/_ro/trn_rl_repo"):
    if os.path.isdir(_p) and _p not in sys.path:
        sys.path.insert(0, _p)

import numpy as np
import ml_dtypes

import concourse.bacc as bacc
import concourse.tile as tile
from concourse import mybir
from concourse.bass_utils import run_bass_kernel_spmd

F32 = mybir.dt.float32
BF16 = mybir.dt.bfloat16
F8 = mybir.dt.float8e4
BF = ml_dtypes.bfloat16
F8NP = ml_dtypes.float8_e4m3fn

N_GRAPHS = 8192
HIDDEN = 256
CORES = 8
WPC = 8            # windows per core
WG = 128           # graphs per window
GRP = 512          # nodes per group
ROW = 258          # xn row: 256 x + 1.0 + pad
EPS = 1e-8
XSCALE = 8.0       # x quantization scale for the mm1 path
W1SCALE = 16.0
W2SCALE = 16.0

import os as _os
HT_FP8 = _os.environ.get("KV_HT_FP8", "0") == "1"
FIRST_CHUNKS = int(_os.environ.get("KV_FIRST_CHUNKS", "1"))
WARM_TABLE = _os.environ.get("KV_WARM", "0") == "1"
FIRST_XN_ACT = _os.environ.get("KV_XN_ACT", "0") == "1"
SEG_LAG = int(_os.environ.get("KV_SEG_LAG", "5"))
MM2_LAG = int(_os.environ.get("KV_MM2_LAG", "3"))
EXPQ = int(_os.environ.get("KV_EXPQ", "2"))
FIN_ACT = _os.environ.get("KV_FIN_ACT", "0") == "1"
BALANCE = _os.environ.get("KV_BALANCE", "1") == "1"
PREFETCH_G = int(_os.environ.get("KV_PREFETCH_G", "2"))
OUT_ACT = _os.environ.get("KV_OUT_ACT", "0") == "1"
INTERLEAVE = _os.environ.get("KV_ILV", "1") == "1"
ALL_XN_ACT = _os.environ.get("KV_ALL_XN_ACT", "0") == "1"
XT2 = _os.environ.get("KV_XT2", "0") == "1"   # prefetch xt 2 windows ahead
OUT16 = _os.environ.get("KV_OUT16", "0") == "1"  # bf16 output stores


def _build_program(T_tiles: int, reps: int = 1, variant: str = "full",
                   zero_bias: bool = False, ht_fp8: bool = HT_FP8):
    n_full = T_tiles // 4
    tail = T_tiles % 4                      # node-tiles in the tail group
    gsz_list = [GRP] * n_full + ([128 * tail] if tail else [])
    ng = len(gsz_list)
    xtbase = [1024 * g for g in range(ng)]  # fp8 elems per partition
    cols = T_tiles
    XNW = T_tiles * ROW
    XTW = 2 * 128 * T_tiles

    nc = bacc.Bacc("TRN2", target_bir_lowering=False, debug=False,
                   num_devices=CORES)
    xn = nc.dram_tensor("xn", [WPC, 128, XNW], BF16, kind="ExternalInput").ap()
    xt = nc.dram_tensor("xt", [WPC, 128, XTW], F8, kind="ExternalInput").ap()
    br = nc.dram_tensor("br", [WPC, 128, cols], F32, kind="ExternalInput").ap()
    # W1 as [mb, kt, m] per partition (contiguous (kt, m) pair blocks for
    # dual-fp8 ldweights); W2 as adjacent (kt) pairs.
    cw1 = nc.dram_tensor("cw1", [128, 2, 2, 128], F8, kind="ExternalInput").ap()
    cw2 = nc.dram_tensor("cw2", [128, 2, 1], F8, kind="ExternalInput").ap()
    cbb = nc.dram_tensor("cbb", [128, 130], BF16, kind="ExternalInput").ap()
    cf = nc.dram_tensor("cf", [128, 3], F32, kind="ExternalInput").ap()
    out = nc.dram_tensor("out", [WPC * WG, HIDDEN], BF16 if OUT16 else F32,
                         kind="ExternalOutput").ap()

    HT_DT = F8 if ht_fp8 else BF16

    from contextlib import ExitStack
    with tile.TileContext(nc) as tc:
        with ExitStack() as ctx:
            cpool = ctx.enter_context(tc.tile_pool(name="const", bufs=1))
            brpool = ctx.enter_context(tc.tile_pool(name="brp", bufs=4 if XT2 else 3))
            xnpool = ctx.enter_context(tc.tile_pool(name="xnp", bufs=3))
            xtpool = ctx.enter_context(tc.tile_pool(name="xtp", bufs=4 if XT2 else 3))
            htpool = ctx.enter_context(tc.tile_pool(name="htp", bufs=6))
            etpool = ctx.enter_context(tc.tile_pool(name="etp", bufs=8))
            spool = ctx.enter_context(tc.tile_pool(name="sp", bufs=24 + 4 * max(0, EXPQ - 2)))
            owpool = ctx.enter_context(tc.tile_pool(name="ow", bufs=2))
            phpool = ctx.enter_context(tc.tile_pool(name="ph", bufs=2, space="PSUM"))
            plpool = ctx.enter_context(tc.tile_pool(name="pl", bufs=2, space="PSUM"))
            pgpool = ctx.enter_context(tc.tile_pool(name="pg", bufs=2, space="PSUM"))
            if reps > 1:
                ctx.enter_context(tc.For_i(0, reps, 1))

            c81 = cpool.tile([128, 2, 2, 128], F8)
            c82 = cpool.tile([128, 2, 1], F8)
            cb = cpool.tile([128, 130], BF16)
            cft = cpool.tile([128, 3], F32)

            def load_consts():
                nc.sync.dma_start(out=c81[:], in_=cw1[:])
                nc.sync.dma_start(out=c82[:], in_=cw2[:])
                nc.sync.dma_start(out=cb[:], in_=cbb[:])
                nc.sync.dma_start(out=cft[:], in_=cf[:])
            iota = cb[:, 0:128]
            if WARM_TABLE:
                warm = cpool.tile([128, 1], F32)
                nc.scalar.activation(warm[:], cft[:, 0:1],
                                     mybir.ActivationFunctionType.Tanh,
                                     bias=0.0, scale=1.0)

            wstate = {}

            def load_xt_br(w):
                brw = brpool.tile([128, cols], F32)
                nc.sync.dma_start(out=brw[:], in_=br[w])
                xtwt = xtpool.tile([128, XTW], F8)
                nc.sync.dma_start(out=xtwt[:], in_=xt[w])
                wstate[w] = dict(brw=brw, xtwt=xtwt, xnc=None)

            def load_xn_w(w, xn_eng=None):
                eng = xn_eng or (nc.scalar if ALL_XN_ACT else nc.sync)
                xnwt = xnpool.tile([128, XNW], BF16)
                eng.dma_start(out=xnwt[:], in_=xn[w])
                pseg = pgpool.tile([128, 257], F32)
                wstate[w].update(xnwt=xnwt, pseg=pseg)

            def load_window(w, xn_chunks=1, xn_eng=None):
                # xt first: mm1 needs it immediately; xn only at seg lag.
                # xn_eng lets window 0's xn ride the (idle) ACT hwdge queue
                # so it streams concurrently with xt on the SP queue.
                load_xt_br(w)
                load_xn_w(w, xn_eng=xn_eng)

            def xn_slice(w, g, t):
                ws = wstate[w]
                base = (g * 4 + t) * ROW
                if ws["xnc"] is not None:
                    step = XNW // len(ws["xnc"])
                    c, off = base // step, base % step
                    return ws["xnc"][c][:, off:off + 257]
                return ws["xnwt"][:, base:base + 257]

            if variant == "dma":
                for w in range(WPC):
                    load_window(w)
                    ws = wstate[w]
                    for nm in ("xnwt", "brw"):
                        dum = etpool.tile([128, 1], F32)
                        nc.vector.tensor_scalar(dum[:], ws[nm][:, 0:1], 1.0,
                                                None, op0=mybir.AluOpType.mult)
                    dum2 = etpool.tile([128, 1], F32)
                    nc.vector.tensor_scalar(dum2[:], ws["xtwt"][:, 0:1],
                                            1.0, None, op0=mybir.AluOpType.mult)

            steps = [] if variant in ("dma", "nop") else \
                [(w, g) for w in range(WPC) for g in range(ng)]
            gstate = {}
            pairstate = {}

            def emit_mm1_mm(i, m):
                w, g = steps[i]
                gsz = gsz_list[g]
                ws = wstate[w]
                if m == 0:
                    ph = phpool.tile([128, 2, GRP], F32)
                    gstate[i] = dict(ph=ph)
                ph = gstate[i]["ph"]
                xtg = ws["xtwt"][:, xtbase[g]:xtbase[g] + 2 * gsz].rearrange(
                    "p (k n) -> p k n", k=2)
                nc.tensor.matmul(ph[:, m, 0:gsz],
                                 c81[:, m],
                                 xtg,
                                 start=True, stop=True,
                                 perf_mode=mybir.MatmulPerfMode.DoubleRow)

            def emit_mm1_tanh(i, mm=True):
                w, g = steps[i]
                gsz = gsz_list[g]
                ws = wstate[w]
                if mm:
                    emit_mm1_mm(i, 0)
                    emit_mm1_mm(i, 1)
                ph = gstate[i]["ph"]
                hsc = 1.0 / (XSCALE * W1SCALE)
                if ht_fp8:
                    # t-major storage so mm2's dual-fp8 lhsT is contiguous
                    ht = htpool.tile([128, 4, 2, 128], HT_DT)
                    ht_w = ht[:].rearrange("p t k n -> p k t n")
                    ph_r = ph[:].rearrange("p k (t n) -> p k t n", t=4)
                    if zero_bias:
                        nc.scalar.activation(ht_w, ph_r,
                                             mybir.ActivationFunctionType.Tanh,
                                             bias=0.0, scale=hsc)
                    else:
                        for m in range(2):
                            nc.scalar.activation(ht_w[:, m], ph_r[:, m],
                                                 mybir.ActivationFunctionType.Tanh,
                                                 bias=cft[:, m:m + 1], scale=hsc)
                else:
                    ht = htpool.tile([128, 2, GRP], HT_DT)
                    if zero_bias:
                        nc.scalar.activation(ht[:, :, 0:gsz], ph[:, :, 0:gsz],
                                             mybir.ActivationFunctionType.Tanh,
                                             bias=0.0, scale=hsc)
                    else:
                        for m in range(2):
                            nc.scalar.activation(ht[:, m, 0:gsz],
                                                 ph[:, m, 0:gsz],
                                                 mybir.ActivationFunctionType.Tanh,
                                                 bias=cft[:, m:m + 1], scale=hsc)
                gstate[i]["ht"] = ht

            def build_s(j, et_ap, base):
                wj, gj = steps[j]
                wsj = wstate[wj]
                sts = []
                for t in range(gsz_list[gj] // 128):
                    st = spool.tile([128, 128], BF16)
                    eng = nc.vector
                    eng.tensor_scalar(st[:], iota,
                                      wsj["brw"][:, gj * 4 + t:gj * 4 + t + 1],
                                      et_ap[:, base + t:base + t + 1],
                                      op0=mybir.AluOpType.is_equal,
                                      op1=mybir.AluOpType.mult)
                    sts.append(st)
                gstate[j]["sts"] = sts

            def emit_mm2_exp_s(i):
                w, g = steps[i]
                gs = gstate[i]
                ht = gs["ht"]
                q = i % EXPQ
                if q == 0:
                    pl = plpool.tile([128, 4 * EXPQ], F32)
                    pairstate[i] = pl
                else:
                    pl = pairstate[i - q]
                    if q == EXPQ - 1:
                        pairstate.pop(i - q)
                lo = 4 * q
                tcnt = gsz_list[g] // 128
                if ht_fp8:
                    for t in range(tcnt):
                        nc.tensor.matmul(pl[:, lo + t:lo + t + 1],
                                         ht[:, t],
                                         c82[:],
                                         start=True, stop=True,
                                         perf_mode=mybir.MatmulPerfMode.DoubleRow)
                else:
                    for t in range(tcnt):
                        for k in range(2):
                            nc.tensor.matmul(pl[:, lo + t:lo + t + 1],
                                             ht[:, k, 128 * t:128 * (t + 1)],
                                             cb[:, 128 + k:129 + k],
                                             start=(k == 0), stop=(k == 1))
                if q == EXPQ - 1 or i == len(steps) - 1:
                    esc = 1.0 / W2SCALE
                    ebias = 0.0 if zero_bias else cft[:, 2:3]
                    et = etpool.tile([128, 4 * EXPQ], F32)
                    span = 4 * q + tcnt
                    nc.scalar.activation(et[:, 0:span], pl[:, 0:span],
                                         mybir.ActivationFunctionType.Exp,
                                         bias=ebias, scale=esc)
                    for j in range(i - q, i + 1):
                        build_s(j, et, 4 * (j - (i - q)))

            def emit_seg(i):
                w, g = steps[i]
                tcnt = gsz_list[g] // 128
                ws = wstate[w]
                gs = gstate.pop(i)
                for t in range(tcnt):
                    nc.tensor.matmul(ws["pseg"][:],
                                     gs["sts"][t][:],
                                     xn_slice(w, g, t),
                                     start=(g == 0 and t == 0),
                                     stop=(g == ng - 1 and t == tcnt - 1))
                if g == ng - 1:
                    finalize_window(w)

            def finalize_window(w):
                ws = wstate.pop(w)
                pseg = ws["pseg"]
                dtmp = owpool.tile([128, 1], F32)
                nc.vector.tensor_scalar_add(dtmp[:], pseg[:, 256:257], EPS)
                rec = owpool.tile([128, 1], F32)
                nc.vector.reciprocal(rec[:], dtmp[:])
                ow = owpool.tile([128, HIDDEN], BF16 if OUT16 else F32)
                if FIN_ACT:
                    nc.scalar.activation(ow[:], pseg[:, 0:256],
                                         mybir.ActivationFunctionType.Copy,
                                         bias=0.0, scale=rec[:])
                else:
                    nc.vector.tensor_scalar(ow[:], pseg[:, 0:256], rec[:],
                                            None, op0=mybir.AluOpType.mult)
                oeng = nc.scalar if OUT_ACT else nc.sync
                oeng.dma_start(out=out[w * WG:(w + 1) * WG, :], in_=ow[:])

            if variant == "nop":
                dnp = owpool.tile([128, 1], F32)
                nc.vector.tensor_scalar(dnp[:], cft[:, 0:1], 1.0, None,
                                        op0=mybir.AluOpType.mult)

            load_consts()
            if steps:
                load_window(0, xn_chunks=FIRST_CHUNKS,
                            xn_eng=nc.scalar if FIRST_XN_ACT else None)
                if XT2 and WPC > 1:
                    load_xt_br(1)
            n = len(steps)
            lag = SEG_LAG
            m2lag = MM2_LAG
            for i in range(n + lag if n else 0):
                if i < n:
                    w, g = steps[i]
                    if g == PREFETCH_G and w + 1 < WPC:
                        if XT2:
                            load_xn_w(w + 1)
                            if w + 2 < WPC:
                                load_xt_br(w + 2)
                        else:
                            load_window(w + 1)
                if INTERLEAVE:
                    if i >= lag:
                        emit_seg(i - lag)
                    if i < n:
                        emit_mm1_mm(i, 0)
                    if m2lag <= i < n + m2lag:
                        emit_mm2_exp_s(i - m2lag)
                    if i < n:
                        emit_mm1_mm(i, 1)
                        emit_mm1_tanh(i, mm=False)
                else:
                    if i >= lag:
                        emit_seg(i - lag)
                    if i < n:
                        emit_mm1_tanh(i)
                    if m2lag <= i < n + m2lag:
                        emit_mm2_exp_s(i - m2lag)
    nc.compile()
    return nc


def _prep_inputs(x, batch, W1, b1, W2, b2):
    batch = np.asarray(batch).astype(np.int64)
    x = np.asarray(x, dtype=np.float32)

    gstarts = np.searchsorted(batch, np.arange(0, N_GRAPHS + 1))
    gsizes = np.diff(gstarts)

    # Assign graphs to windows within each core, balancing node counts
    # (LPT greedy) so the global max window -- and with it T -- shrinks.
    perm = np.zeros((CORES, WPC, WG), np.int64)
    wmax = 0
    for c in range(CORES):
        ids = np.arange(c * WPC * WG, (c + 1) * WPC * WG)
        if BALANCE:
            order = ids[np.argsort(-gsizes[ids], kind="stable")]
            loads = [0] * WPC
            counts = [0] * WPC
            buckets = [[] for _ in range(WPC)]
            for gid in order:
                cand = [i for i in range(WPC) if counts[i] < WG]
                wsel = min(cand, key=lambda i: loads[i])
                buckets[wsel].append(gid)
                loads[wsel] += int(gsizes[gid])
                counts[wsel] += 1
            for w in range(WPC):
                perm[c, w] = np.sort(np.array(buckets[w], np.int64))
            wmax = max(wmax, max(loads))
        else:
            perm[c] = ids.reshape(WPC, WG)
            for w in range(WPC):
                wmax = max(wmax, int(gsizes[perm[c, w]].sum()))

    T_tiles = max(4, (int(wmax) + 127) // 128)
    n_full = T_tiles // 4
    tail = T_tiles % 4
    gsz_list = [GRP] * n_full + ([128 * tail] if tail else [])
    T = T_tiles * 128
    cols = T_tiles
    XNW = T_tiles * ROW
    XTW = 2 * 128 * T_tiles

    xbf = x.astype(BF)
    x8 = (x * XSCALE).astype(F8NP)

    W1 = np.asarray(W1, np.float32)
    W2 = np.asarray(W2, np.float32).reshape(-1)
    cw1 = np.zeros((128, 2, 2, 128), dtype=F8NP)
    cw2 = np.zeros((128, 2, 1), dtype=F8NP)
    for kt in range(2):
        for mb in range(2):
            cw1[:, mb, kt, :] = (W1[kt * 128:(kt + 1) * 128,
                                    mb * 128:(mb + 1) * 128] * W1SCALE).astype(F8NP)
        cw2[:, kt, 0] = (W2[kt * 128:(kt + 1) * 128] * W2SCALE).astype(F8NP)
    cbb = np.zeros((128, 130), dtype=BF)
    cbb[:, 0:128] = np.tile(np.arange(128, dtype=np.float32), (128, 1)).astype(BF)
    for k in range(2):
        cbb[:, 128 + k] = (W2[128 * k:128 * (k + 1)] * W2SCALE).astype(BF)

    cf = np.zeros((128, 3), dtype=np.float32)
    cf[:, 0] = np.asarray(b1, np.float32)[0:128]
    cf[:, 1] = np.asarray(b1, np.float32)[128:256]
    cf[:, 2] = float(np.asarray(b2, np.float32).reshape(-1)[0])
    zero_bias = bool((np.asarray(b1) == 0).all() and (np.asarray(b2) == 0).all())

    in_maps = []
    for c in range(CORES):
        xn_c = np.zeros((WPC, T, ROW), dtype=BF)
        xt_lin = np.zeros((WPC, 2, 128, T), dtype=F8NP)
        br_c = np.full((WPC, 128, cols), -1.0, dtype=np.float32)
        for w in range(WPC):
            gl = perm[c, w]
            sz = int(gsizes[gl].sum())
            if sz:
                idx = np.concatenate(
                    [np.arange(gstarts[g], gstarts[g + 1]) for g in gl])
                xn_c[w, :sz, 0:256] = xbf[idx]
                xn_c[w, :sz, 256] = BF(1.0)
                xt_lin[w, 0, :, :sz] = x8[idx, 0:128].T
                xt_lin[w, 1, :, :sz] = x8[idx, 128:256].T
                tmp = np.full(T, -1.0, dtype=np.float32)
                tmp[:sz] = np.repeat(
                    np.arange(WG, dtype=np.float32), gsizes[gl])
                br_c[w] = tmp.reshape(cols, 128).T
        # xn swizzle: [w, tile*128+p, d] -> [w, p, tile*ROW + d]
        xn_sw = np.ascontiguousarray(
            xn_c.reshape(WPC, T_tiles, 128, ROW).transpose(0, 2, 1, 3)
        ).reshape(WPC, 128, XNW)
        # xt flat: per group [kt, 128, gsz] -> [128, 2*gsz] at base 1024*g
        xt_sw = np.zeros((WPC, 128, XTW), dtype=F8NP)
        for g, gsz in enumerate(gsz_list):
            s0 = g * GRP
            blk = xt_lin[:, :, :, s0:s0 + gsz]          # [WPC, 2, 128, gsz]
            xt_sw[:, :, 1024 * g:1024 * g + 2 * gsz] = np.ascontiguousarray(
                blk.transpose(0, 2, 1, 3)).reshape(WPC, 128, 2 * gsz)
        in_maps.append(dict(xn=xn_sw, xt=xt_sw, br=br_c, cw1=cw1, cw2=cw2,
                            cbb=cbb, cf=cf))
    return T_tiles, in_maps, zero_bias, perm


_PROGRAM_CACHE = {}


def kernel(x, batch, W1, b1, W2, b2):
    T_tiles, in_maps, zb, perm = _prep_inputs(x, batch, W1, b1, W2, b2)
    key = (T_tiles, zb)
    if key not in _PROGRAM_CACHE:
        _PROGRAM_CACHE[key] = _build_program(T_tiles, zero_bias=zb)
    nc = _PROGRAM_CACHE[key]
    res = run_bass_kernel_spmd(nc, in_maps, list(range(CORES))).results
    raw = np.concatenate([res[c]["out"] for c in range(CORES)], axis=0)
    final = np.empty(raw.shape, np.float32)
    final[perm.reshape(-1)] = raw
    return final

